# revision 18
# baseline (speedup 1.0000x reference)
"""Trainium2 Bass kernel for an encoder-decoder (S2S) transformer.

Distribution: 8 NeuronCores = 4 data-parallel groups (batch B=4) x 2-way
Megatron tensor-parallel within each same-SEngine core pair.  Per TP pair,
attention heads (qkv/out) and ffn (ff1/ff2) are sharded; partials combine
with a 2-core AllReduce after the attention out-projection and after ff2.

Matmuls run in bf16 on the TensorEngine (fp32 PSUM accumulation); the
residual stream and layernorm statistics stay fp32.  Activations are
SBUF-resident for the whole pass; only weights stream from HBM.

Pipelining structure (v2):
  - AllReduce at 1-tile granularity, overlapped with the tail of the
    producing projection and the head of the consuming one.
  - natural->transposed layout conversion via XBAR DMA transposes from a
    bf16 mirror of the residual stream (no PE transposes).
  - LayerNorm stats via bn_stats/bn_aggr; rstd = exp(-0.5*ln(var+eps)) so
    the ACT engine stays on one activation-table set (ln/exp/relu/copy).
  - softmax denominator via reciprocal_approx_fast.
  - the first projection after each LN boundary consumes x_T in two
    256-token windows so it can start before the last tiles clear the
    AllReduce.
  - out-proj/ff2 stream k-chunks (t-inner loops) so they consume
    attn_T/h_T incrementally.

Layout conventions (per core, P=128):
  x_nat[t]  [P, D]   fp32   natural tokens-on-partitions residual stream
  xb[t]     [P, D]   bf16   mirror of x_nat (source for DMA transposes)
  x_T       [P, ND*T] bf16  transposed: chunk d cols [d*T:(d+1)*T]
  q_T/k_T   [P, NQK*T] bf16 rows = sharded head dims, chunk m = heads 2m,2m+1
  v_aug     [P, NT*WA] bf16 t-chunk: per head 64 value cols + 64 ones cols
  attn_T    [P, NO*T] bf16  context, transposed (rows = sharded dims)
  h_T       [P, NFF*T] bf16 ffn hidden, transposed
"""

import os
import sys

for _p in ("/opt/trn_rl_repo", "/root/.axon_site/_ro/trn_rl_repo"):
    if os.path.isdir(_p) and _p not in sys.path:
        sys.path.insert(0, _p)

import numpy as np
import ml_dtypes

import concourse.bass as bass
import concourse.bacc as bacc
import concourse.tile as tile
from concourse import mybir
from concourse.bass import IndirectOffsetOnAxis

BF16 = ml_dtypes.bfloat16
F32 = mybir.dt.float32
BF = mybir.dt.bfloat16
I32 = mybir.dt.int32
AF = mybir.ActivationFunctionType
ALU = mybir.AluOpType
AX = mybir.AxisListType

P = 128


class Cfg:
    def __init__(self, B, Q, T, D, H, V, L, FF, TP, n_cores, flags=frozenset()):
        self.B, self.Q, self.T, self.D, self.H, self.V, self.L, self.FF = \
            B, Q, T, D, H, V, L, FF
        self.TP, self.n_cores = TP, n_cores
        self.E = D // Q
        self.HD = D // H
        assert self.HD == 64, "head packing assumes head_dim 64"
        assert self.E == P, "per-quantizer embedding dim must be 128"
        self.SCALE = 1.0 / float(np.sqrt(self.HD))
        self.DL = D // TP
        self.FFL = FF // TP
        self.HL = H // TP
        self.NT = T // P
        self.ND = D // P
        self.NQK = self.DL // P
        self.NO = self.DL // P      # attn_T chunks
        self.NFF = self.FFL // P
        assert self.HL % 2 == 0, "needs an even number of local heads"
        self.flags = frozenset(flags)

    def key(self):
        return (self.B, self.Q, self.T, self.D, self.H, self.V, self.L,
                self.FF, self.TP, self.n_cores, tuple(sorted(self.flags)))


# --------------------------------------------------------------------------
# program builder
# --------------------------------------------------------------------------

def build_program(c: Cfg):
    nc = bacc.Bacc(None, target_bir_lowering=False, num_devices=c.n_cores)

    def din(name, shape, dt=BF):
        return nc.dram_tensor(name, shape, dt, kind="ExternalInput")

    codes_in = din("codes_in", [c.Q, c.T], I32)
    codes_tgt = din("codes_tgt", [c.Q, c.T], I32)
    tok_emb = [din(f"tok_emb_{q}", [c.V, c.E], F32) for q in range(c.Q)]
    pos = din("pos", [c.T, c.D], F32)

    w = {}
    for l in range(c.L):
        for nm, sh in [
                # qk packs: [P, (2*NQK) * (ND*P)]; v row-major [D, DL]
                (f"e_qkv_{l}", [P, 2 * c.NQK * c.D]),
                (f"e_v_{l}", [c.D, c.DL]),
                (f"e_out_{l}", [c.DL, c.D]),
                (f"e_ff1_{l}", [P, c.NFF * c.D]),
                (f"e_ff2_{l}", [c.FFL, c.D]),
                (f"d_sqkv_{l}", [P, 2 * c.NQK * c.D]),
                (f"d_sv_{l}", [c.D, c.DL]),
                (f"d_sout_{l}", [c.DL, c.D]),
                (f"d_cqkv_{l}", [P, 2 * c.NQK * c.D]),
                (f"d_cv_{l}", [c.D, c.DL]),
                (f"d_cout_{l}", [c.DL, c.D]),
                (f"d_ff1_{l}", [P, c.NFF * c.D]),
                (f"d_ff2_{l}", [c.FFL, c.D])]:
            w[nm] = din(nm, sh)
    w["head_t"] = din("head_t", [c.E, c.Q * c.V])

    opt = {}
    for nm in c.flags:
        if "_qkv_b_" in nm or "_sqkv_b_" in nm or "_cqkv_b_" in nm:
            opt[nm] = din(nm, [3 * c.DL], F32)
        elif "_ff1_b_" in nm:
            opt[nm] = din(nm, [c.FFL], F32)
        elif nm == "head_b":
            opt[nm] = din(nm, [P, c.Q * c.V], F32)
        else:
            opt[nm] = din(nm, [P, c.D], F32)

    logits = nc.dram_tensor("logits", [c.Q, c.T, c.V], BF,
                            kind="ExternalOutput")

    groups = ([[g * c.TP + i for i in range(c.TP)]
               for g in range(c.n_cores // c.TP)] if c.TP > 1 else None)

    with tile.TileContext(nc) as tc:
        _emit(nc, tc, c, codes_in, codes_tgt, tok_emb, pos, w, opt, logits,
              groups)
    nc.compile()
    return nc


def _emit(nc, tc, c, codes_in, codes_tgt, tok_emb, pos, w, opt, logits,
          groups):
    from contextlib import ExitStack
    es = ExitStack()
    pool = lambda name, bufs, space="SBUF": es.enter_context(
        tc.tile_pool(name=name, bufs=bufs, space=space))

    XTR_MODE = os.environ.get("BASS_S2S_XTR", "batch")
    SPLIT_ON = os.environ.get("BASS_S2S_SPLIT", "1") == "1" and \
        XTR_MODE != "pe"
    AR_GRAN = int(os.environ.get("BASS_S2S_ARG", "1"))

    const = pool("const", 1)
    persist = pool("persist", 1)
    wpool = pool("wpool", 3)
    act = pool("act", 3)
    scratch = pool("scratch", 2)
    dram = pool("dram", 4, space="DRAM")
    ps_proj = pool("ps_proj", 4 if SPLIT_ON else 3, space="PSUM")
    ps_s = pool("ps_s", 2 if XTR_MODE != "pe" else 1, space="PSUM")
    ps_av = pool("ps_av", 2, space="PSUM")
    ps_tr = pool("ps_tr", 2, space="PSUM") if XTR_MODE == "pe" else None

    # constants
    if XTR_MODE == "pe":
        ident_b = const.tile([P, P], BF, name="ident_b")
        from concourse.masks import make_identity
        make_identity(nc, ident_b[:])
    causT = const.tile([P, P], F32, name="causT")
    nc.gpsimd.memset(causT[:], 0.0)
    # transposed causal: fill -1e9 where tk > tq  (iota = tq - tk >= 0 keeps)
    nc.gpsimd.affine_select(out=causT[:], in_=causT[:],
                            compare_op=ALU.is_ge, fill=-1e9, base=0,
                            pattern=[[1, P]], channel_multiplier=-1)

    eps_t = const.tile([P, 1], F32, name="eps_t")
    nc.vector.memset(eps_t[:], 1e-5)

    opt_sb = {}
    for nm in opt:
        if "qkv_b_" in nm:
            t = const.tile([P, 3 * c.NQK], F32, name=f"sb_{nm}")
            nc.sync.dma_start(out=t[:],
                              in_=opt[nm].rearrange("(m p) -> p m", p=P))
        elif "_ff1_b_" in nm:
            t = const.tile([P, c.NFF], F32, name=f"sb_{nm}")
            nc.sync.dma_start(out=t[:],
                              in_=opt[nm].rearrange("(m p) -> p m", p=P))
        else:
            sh = [P, c.Q * c.V] if nm == "head_b" else [P, c.D]
            t = const.tile(sh, F32, name=f"sb_{nm}")
            nc.sync.dma_start(out=t[:], in_=opt[nm][:])
        opt_sb[nm] = t

    # persistent activations
    x_nat = [persist.tile([P, c.D], F32, name=f"x_{t}", tag=f"x_{t}")
             for t in range(c.NT)]
    y_nat = [persist.tile([P, c.D], F32, name=f"y_{t}", tag=f"y_{t}")
             for t in range(c.NT)]
    xb = [persist.tile([P, c.D], BF, name=f"xb_{t}", tag=f"xb_{t}")
          for t in range(c.NT)]
    x_T = persist.tile([P, c.ND * c.T], BF, name="x_T", tag="x_T")
    mem_T = persist.tile([P, c.ND * c.T], BF, name="mem_T", tag="mem_T")
    q_T = persist.tile([P, c.NQK * c.T], BF, name="q_T", tag="q_T")
    k_T = persist.tile([P, c.NQK * c.T], BF, name="k_T", tag="k_T")
    WA = c.HL * 128   # per-t block: per head 64 value cols + 64 ones cols
    v_aug = persist.tile([P, c.NT * WA], BF, name="v_aug", tag="v_aug")
    attn_T = persist.tile([P, c.NO * c.T], BF, name="attn_T", tag="attn_T")
    h_T = persist.tile([P, c.NFF * c.T], BF, name="h_T", tag="h_T")

    # ---------------- helpers ----------------
    def mirror_transpose(cur, t, dst_T):
        """xb[t] = bf16(cur[t]); dst_T[:, d*T+t*P:+P] = xb[t][:, dP:(d+1)P].T
        via XBAR DMA transposes (no PE involvement).  The batched form
        transposes the whole [P, D] tile in one instruction: logical row
        j = d*P + p of the transposed tile lands at partition p, chunk d."""
        nc.scalar.copy(out=xb[t][:], in_=cur[t][:])
        if XTR_MODE == "pe":
            for d in range(c.ND):
                pt = ps_tr.tile([P, P], BF, tag="ps_tr", name="pt")
                nc.tensor.transpose(pt[:], xb[t][:, d * P:(d + 1) * P],
                                    ident_b[:])
                dsl = dst_T[:, d * c.T + t * P: d * c.T + t * P + P]
                if (t + d) % 2 == 0:
                    nc.scalar.copy(out=dsl, in_=pt[:])
                else:
                    nc.vector.tensor_copy(out=dsl, in_=pt[:])
        elif XTR_MODE == "chunk":
            for d in range(c.ND):
                eng = nc.sync if (t + d) % 2 == 0 else nc.scalar
                eng.dma_start_transpose(
                    out=dst_T[:, d * c.T + t * P: d * c.T + t * P + P],
                    in_=xb[t][:, d * P:(d + 1) * P])
        else:
            view = dst_T[:].rearrange("p (d tt) -> p d tt", d=c.ND)[
                :, :, t * P:(t + 1) * P]
            eng = nc.sync if t % 2 == 0 else nc.scalar
            eng.dma_start_transpose(out=view, in_=xb[t][:])

    def embed(codes, cur, dst_T):
        """Gather token embeddings + positional into cur (fp32); when dst_T
        is given, also produce the bf16 mirror + transposed copy."""
        for t in range(c.NT):
            ptile = scratch.tile([P, c.D], F32, tag="pos", name="ptile",
                                 bufs=1)
            nc.sync.dma_start(out=ptile[:], in_=pos[t * P:(t + 1) * P, :])
            for q in range(c.Q):
                idx = scratch.tile([P, 1], I32, tag="idx", name="idx")
                nc.sync.dma_start(out=idx[:], in_=codes[q, t * P:(t + 1) * P])
                nc.gpsimd.indirect_dma_start(
                    out=cur[t][:, q * c.E:(q + 1) * c.E],
                    out_offset=None,
                    in_=tok_emb[q][:],
                    in_offset=IndirectOffsetOnAxis(ap=idx[:, :1], axis=0))
            nc.vector.tensor_tensor(out=cur[t][:], in0=cur[t][:],
                                    in1=ptile[:], op=ALU.add)
            if dst_T is not None:
                mirror_transpose(cur, t, dst_T)

    def ln_tile(cur, t, sub, gname, bname, dst_T):
        """cur[t] = LN(cur[t] + sub) (post-norm), then mirror+transpose."""
        xt = cur[t]
        nc.vector.tensor_tensor(out=xt[:], in0=xt[:], in1=sub[:], op=ALU.add)
        if os.environ.get("BASS_S2S_BN", "1") == "1":
            stats = scratch.tile([P, 2, 6], F32, tag="lnstat", name="stats",
                                 bufs=8)
            half = c.D // 2
            nc.vector.bn_stats(stats[:, 0:1, :], xt[:, 0:half])
            nc.vector.bn_stats(stats[:, 1:2, :], xt[:, half:c.D])
            aggr = scratch.tile([P, 2], F32, tag="lnstat", name="aggr",
                                bufs=8)
            nc.vector.bn_aggr(aggr[:], stats[:])
            mean_ap, var_ap = aggr[:, 0:1], aggr[:, 1:2]
        else:
            s1 = scratch.tile([P, 1], F32, tag="lnstat", name="s1", bufs=8)
            s2 = scratch.tile([P, 1], F32, tag="lnstat", name="s2", bufs=8)
            sq = scratch.tile([P, c.D], BF, tag="lnsq", name="sq", bufs=1)
            nc.vector.reduce_sum(out=s1[:], in_=xt[:], axis=AX.X)
            nc.scalar.activation(sq[:], xt[:], AF.Square, accum_out=s2[:])
            mean = scratch.tile([P, 1], F32, tag="lnstat", name="mean",
                                bufs=8)
            var = scratch.tile([P, 1], F32, tag="lnstat", name="var", bufs=8)
            m2 = scratch.tile([P, 1], F32, tag="lnstat", name="m2", bufs=8)
            nc.vector.tensor_scalar_mul(mean[:], s1[:], 1.0 / c.D)
            nc.vector.tensor_scalar_mul(var[:], s2[:], 1.0 / c.D)
            nc.vector.tensor_tensor(out=m2[:], in0=mean[:], in1=mean[:],
                                    op=ALU.mult)
            nc.vector.tensor_tensor(out=var[:], in0=var[:], in1=m2[:],
                                    op=ALU.subtract)
            mean_ap, var_ap = mean[:], var[:]
        # rstd = 1/sqrt(var + eps).  Sqrt shares a table set with the
        # square/relu/copy fillers; Ln/Exp alternation would thrash ACT
        # table loads.
        rstd = scratch.tile([P, 1], F32, tag="lnstat", name="rstd", bufs=8)
        nc.scalar.activation(rstd[:], var_ap, AF.Sqrt, bias=eps_t[:])
        nc.vector.reciprocal(rstd[:], rstd[:])
        nc.vector.tensor_scalar(out=xt[:], in0=xt[:], scalar1=mean_ap,
                                scalar2=rstd[:], op0=ALU.subtract,
                                op1=ALU.mult)
        if gname in opt_sb:
            nc.vector.tensor_tensor(out=xt[:], in0=xt[:],
                                    in1=opt_sb[gname][:], op=ALU.mult)
        if bname in opt_sb:
            nc.vector.tensor_tensor(out=xt[:], in0=xt[:],
                                    in1=opt_sb[bname][:], op=ALU.add)
        mirror_transpose(cur, t, dst_T)

    def ar_ln(cur, parts, gname, bname, dst_T):
        """AllReduce sublayer partials over the TP pair at 1-tile
        granularity (pipelined), then per-tile residual add + layernorm."""
        if groups is None:
            for t in range(c.NT):
                ln_tile(cur, t, parts[t], gname, bname, dst_T)
            return
        G = AR_GRAN
        for t0 in range(0, c.NT, G):
            n = min(G, c.NT - t0)
            arin = dram.tile([n * P, c.D], BF, tag="arin", name="arin")
            arout = dram.tile([n * P, c.D], BF, tag="arout", name="arout")
            for i in range(n):
                nc.sync.dma_start(out=arin[i * P:(i + 1) * P, :],
                                  in_=parts[t0 + i][:])
            nc.gpsimd.collective_compute(
                "AllReduce", ALU.add, replica_groups=groups,
                ins=[arin[:].opt()], outs=[arout[:].opt()])
            for i in range(n):
                red = scratch.tile([P, c.D], BF, tag="ar_red", name="red")
                nc.sync.dma_start(out=red[:], in_=arout[i * P:(i + 1) * P, :])
                ln_tile(cur, t0 + i, red, gname, bname, dst_T)

    def proj_qk(wname, bname, src_T, m_base, dst, split):
        """dst[:, m*T:(m+1)*T] = rows [m_base+m]*P.. of W.T @ src.
        m_base indexes 1024-col chunks of the packed weight tensor.
        split=True consumes src_T in two 256-token windows so the first
        window can start before the last LN tiles land."""
        wts, pss = [], []
        for m in range(c.NQK):
            mg = m_base + m
            wt = wpool.tile([P, c.ND * P], BF, tag="wqkv", name="wt")
            nc.sync.dma_start(out=wt[:],
                              in_=w[wname][:, mg * c.D:(mg + 1) * c.D])
            wts.append(wt)
        windows = [(0, c.T)] if not split else [(0, c.T // 2),
                                               (c.T // 2, c.T)]
        if split:
            pss = [ps_proj.tile([P, c.T], F32, tag="ps_proj", name="ps")
                   for _ in range(c.NQK)]
        for wi, (w0, w1) in enumerate(windows):
            for m in range(c.NQK):
                mg = m_base + m
                if split:
                    ps = pss[m]
                else:
                    ps = ps_proj.tile([P, c.T], F32, tag="ps_proj", name="ps")
                for k in range(c.ND):
                    nc.tensor.matmul(ps[:, w0:w1],
                                     wts[m][:, k * P:(k + 1) * P],
                                     src_T[:, k * c.T + w0: k * c.T + w1],
                                     start=(k == 0), stop=(k == c.ND - 1))
                col = m * c.T
                if bname in opt_sb:
                    nc.vector.tensor_scalar(
                        out=dst[:, col + w0:col + w1], in0=ps[:, w0:w1],
                        scalar1=opt_sb[bname][:, mg:mg + 1], scalar2=None,
                        op0=ALU.add)
                else:
                    eng = nc.vector.tensor_copy if m % 2 == 0 else \
                        (lambda out, in_: nc.scalar.copy(out=out, in_=in_))
                    eng(out=dst[:, col + w0:col + w1], in_=ps[:, w0:w1])

    def proj_v(wname, src_T):
        wts = []
        for k in range(c.ND):
            wt = wpool.tile([P, c.DL], BF, tag=f"wv_{k}", name="wt", bufs=1)
            nc.sync.dma_start(out=wt[:], in_=w[wname][k * P:(k + 1) * P, :])
            wts.append(wt)
        for t in range(c.NT):
            ps = ps_proj.tile([P, c.DL], F32, tag="ps_proj", name="ps")
            for k in range(c.ND):
                nc.tensor.matmul(
                    ps[:], src_T[:, k * c.T + t * P: k * c.T + t * P + P],
                    wts[k][:], start=(k == 0), stop=(k == c.ND - 1))
            blk = v_aug[:, t * WA:(t + 1) * WA].rearrange(
                "p (h f) -> p h f", f=128)
            nc.scalar.copy(out=blk[:, :, 0:64],
                           in_=ps[:].rearrange("p (h f) -> p h f", f=64))

    # ones columns of v_aug are set once; projections only write data cols
    for t in range(c.NT):
        blk = v_aug[:, t * WA:(t + 1) * WA].rearrange("p (h f) -> p h f", f=128)
        nc.vector.memset(blk[:, :, 64:128], 1.0)

    def attention(causal):
        """q_T,k_T,v_aug -> attn_T.  Scores are computed pre-transposed
        (S^T[tk, tq] = k_blk^T q) so no A transposes are needed; the softmax
        denominator rides along as a ones column in v_aug and the division
        happens on the small [64, T] context output."""
        for h in range(c.HL):
            m = h // 2
            po = 64 * (h % 2)
            at_tiles = [act.tile([P, c.T], BF, tag=f"AT{tk}", name="at",
                                 bufs=2) for tk in range(c.NT)]
            for tk in range(c.NT):
                tq0 = tk * P if causal else 0
                pss = ps_s.tile([P, c.T], F32, tag="ps_s", name="pss")
                nc.tensor.matmul(
                    pss[:, tq0:c.T],
                    k_T[po:po + 64, m * c.T + tk * P: m * c.T + tk * P + P],
                    q_T[po:po + 64, m * c.T + tq0: m * c.T + c.T],
                    start=True, stop=True)
                if causal:
                    nc.vector.tensor_tensor(
                        out=pss[:, tk * P:(tk + 1) * P],
                        in0=pss[:, tk * P:(tk + 1) * P],
                        in1=causT[:], op=ALU.add)
                nc.scalar.activation(at_tiles[tk][:, tq0:c.T],
                                     pss[:, tq0:c.T], AF.Exp, scale=c.SCALE)
            ps_o = ps_av.tile([P, c.T], F32, tag="ps_av", name="ps_o")
            for tk in range(c.NT):
                cols0 = tk * P if causal else 0
                nc.tensor.matmul(
                    ps_o[:, cols0:c.T],
                    v_aug[:, tk * WA + h * 128: tk * WA + h * 128 + 128],
                    at_tiles[tk][:, cols0:c.T],
                    start=(tk == 0), stop=(tk == c.NT - 1))
            rden = scratch.tile([64, c.T], F32, tag="rden", name="rden",
                                bufs=2)
            if os.environ.get("BASS_S2S_RAF", "1") == "1":
                # reciprocal_approx_fast misbehaves on PSUM inputs on HW
                # (bit-trick seed); bounce the denominator through SBUF.
                den = scratch.tile([64, c.T], F32, tag="rden", name="den",
                                   bufs=2)
                nc.scalar.copy(out=den[:], in_=ps_o[64:128, :])
                nc.vector.reciprocal_approx_fast(out=rden[:], in_=den[:])
            else:
                nc.vector.reciprocal(rden[:], ps_o[64:128, :])
            nc.vector.tensor_tensor(
                out=attn_T[po:po + 64, m * c.T:(m + 1) * c.T],
                in0=ps_o[0:64, :], in1=rden[:], op=ALU.mult)

    NB = max(c.D // 512, 1)
    NW = min(512, c.D)

    def mm_to_natural(src_T, nk, wts, bname):
        """[T, D] = src_T.T @ W, returned as per-t bf16 [P, D] tiles.
        Emitted half-by-half (t01 then t23) with t-inner k-streaming so
        the AllReduce of early tiles overlaps the tail, and src_T chunks
        are consumed as soon as the producing stage writes them."""
        parts = [scratch.tile([P, c.D], BF, tag="oproj", name=f"sb{t}",
                              bufs=3) for t in range(c.NT)]
        for half in range(2):
            ts = (0, 1) if half == 0 else (2, 3)
            for n in range(NB):
                pss = {t: ps_proj.tile([P, NW], F32, tag="ps_proj",
                                       name="ps") for t in ts}
                for k in range(nk):
                    for t in ts:
                        nc.tensor.matmul(
                            pss[t][:],
                            src_T[:, k * c.T + t * P: k * c.T + t * P + P],
                            wts[k][:, n * NW:(n + 1) * NW],
                            start=(k == 0), stop=(k == nk - 1))
                for t in ts:
                    dsl = parts[t][:, n * NW:(n + 1) * NW]
                    if t % 2 == 0:
                        nc.vector.tensor_copy(out=dsl, in_=pss[t][:])
                    else:
                        nc.scalar.copy(out=dsl, in_=pss[t][:])
            for t in ts:
                if bname in opt_sb:
                    nc.vector.tensor_tensor(out=parts[t][:], in0=parts[t][:],
                                            in1=opt_sb[bname][:], op=ALU.add)
        return parts

    def out_proj(wname, bname):
        wts = []
        for k in range(c.NO):
            wt = wpool.tile([P, c.D], BF, tag=f"wo_{k}", name="wt", bufs=1)
            nc.sync.dma_start(out=wt[:], in_=w[wname][k * P:(k + 1) * P, :])
            wts.append(wt)
        return mm_to_natural(attn_T, c.NO, wts, bname)

    def ffn1(w1name, b1name, split_first):
        """h_T = relu(W1.T @ x); first `split_first` m-chunks consume x_T
        in two windows so they can start before the LN tail lands."""
        def emit_m(m, wt, ps, w0, w1):
            for k in range(c.ND):
                nc.tensor.matmul(ps[:, w0:w1], wt[:, k * P:(k + 1) * P],
                                 x_T[:, k * c.T + w0: k * c.T + w1],
                                 start=(k == 0), stop=(k == c.ND - 1))
            if b1name in opt_sb:
                nc.scalar.activation(h_T[:, m * c.T + w0:m * c.T + w1],
                                     ps[:, w0:w1], AF.Relu,
                                     bias=opt_sb[b1name][:, m:m + 1])
            else:
                nc.scalar.activation(h_T[:, m * c.T + w0:m * c.T + w1],
                                     ps[:, w0:w1], AF.Relu)

        wts = {}
        for m in range(c.NFF):
            wt = wpool.tile([P, c.ND * P], BF, tag="wff1", name="wt",
                            bufs=6)
            nc.sync.dma_start(out=wt[:],
                              in_=w[w1name][:, m * c.D:(m + 1) * c.D])
            wts[m] = wt
        sp = min(split_first, 4)
        pss = [ps_proj.tile([P, c.T], F32, tag="ps_proj", name="ps")
               for _ in range(sp)]
        for m in range(sp):
            emit_m(m, wts[m], pss[m], 0, c.T // 2)
        for m in range(sp):
            emit_m(m, wts[m], pss[m], c.T // 2, c.T)
        for m in range(sp, c.NFF):
            ps = ps_proj.tile([P, c.T], F32, tag="ps_proj", name="ps")
            emit_m(m, wts[m], ps, 0, c.T)

    def ffn2(w2name, b2name):
        wts = []
        for k in range(c.NFF):
            wt = wpool.tile([P, c.D], BF, tag=f"wff2_{k}", name="wt", bufs=1)
            nc.sync.dma_start(out=wt[:], in_=w[w2name][k * P:(k + 1) * P, :])
            wts.append(wt)
        return mm_to_natural(h_T, c.NFF, wts, b2name)

    # ---------------- encoder ----------------
    embed(codes_in, x_nat, x_T)
    # prefetch decoder embeddings; gathers overlap encoder compute
    embed(codes_tgt, y_nat, None)
    for l in range(c.L):
        proj_qk(f"e_qkv_{l}", f"e_qkv_b_{l}", x_T, 0, q_T, split=SPLIT_ON)
        proj_qk(f"e_qkv_{l}", f"e_qkv_b_{l}", x_T, c.NQK, k_T, split=False)
        proj_v(f"e_v_{l}", x_T)
        attention(causal=False)
        ar_ln(x_nat, out_proj(f"e_out_{l}", f"e_out_b_{l}"),
              f"e_ln1_w_{l}", f"e_ln1_b_{l}", x_T)
        ffn1(f"e_ff1_{l}", f"e_ff1_b_{l}", split_first=4 if SPLIT_ON else 0)
        dst = mem_T if l == c.L - 1 else x_T
        ar_ln(x_nat, ffn2(f"e_ff2_{l}", f"e_ff2_b_{l}"),
              f"e_ln2_w_{l}", f"e_ln2_b_{l}", dst)

    # ---------------- decoder ----------------
    for t in range(c.NT):
        mirror_transpose(y_nat, t, x_T)
    for l in range(c.L):
        proj_qk(f"d_sqkv_{l}", f"d_sqkv_b_{l}", x_T, 0, q_T, split=SPLIT_ON)
        proj_qk(f"d_sqkv_{l}", f"d_sqkv_b_{l}", x_T, c.NQK, k_T, split=False)
        proj_v(f"d_sv_{l}", x_T)
        attention(causal=True)
        _sp = out_proj(f"d_sout_{l}", f"d_sout_b_{l}")
        # cross-attention K/V depend only on encoder memory -- emitted here
        # so the TensorEngine stays busy while the AllReduce is in flight
        proj_qk(f"d_cqkv_{l}", f"d_cqkv_b_{l}", mem_T, c.NQK, k_T,
                split=False)
        proj_v(f"d_cv_{l}", mem_T)
        ar_ln(y_nat, _sp, f"d_ln1_w_{l}", f"d_ln1_b_{l}", x_T)
        proj_qk(f"d_cqkv_{l}", f"d_cqkv_b_{l}", x_T, 0, q_T, split=SPLIT_ON)
        attention(causal=False)
        ar_ln(y_nat, out_proj(f"d_cout_{l}", f"d_cout_b_{l}"),
              f"d_ln2_w_{l}", f"d_ln2_b_{l}", x_T)
        ffn1(f"d_ff1_{l}", f"d_ff1_b_{l}", split_first=4 if SPLIT_ON else 0)
        ar_ln(y_nat, ffn2(f"d_ff2_{l}", f"d_ff2_b_{l}"),
              f"d_ln3_w_{l}", f"d_ln3_b_{l}", x_T)

    # ---------------- output head (all Q on every core) ----------------
    hws = []
    for j in range(c.Q):
        hw = wpool.tile([P, c.V], BF, tag=f"whead_{j}", name="hw", bufs=1)
        nc.sync.dma_start(out=hw[:],
                          in_=w["head_t"][:, j * c.V:(j + 1) * c.V])
        hws.append(hw)
    NBV = max(c.V // 512, 1)
    NWV = min(512, c.V)
    for t in range(c.NT):
        for j in range(c.Q):
            sb = scratch.tile([P, c.V], BF, tag="lgt", name="sb", bufs=2)
            for n in range(NBV):
                ps = ps_proj.tile([P, NWV], F32, tag="ps_proj", name="ps")
                nc.tensor.matmul(
                    ps[:], x_T[:, j * c.T + t * P: j * c.T + t * P + P],
                    hws[j][:, n * NWV:(n + 1) * NWV], start=True, stop=True)
                dsl = sb[:, n * NWV:(n + 1) * NWV]
                if (t + j + n) % 2 == 0:
                    nc.vector.tensor_copy(out=dsl, in_=ps[:])
                else:
                    nc.scalar.copy(out=dsl, in_=ps[:])
            if "head_b" in opt_sb:
                nc.vector.tensor_tensor(
                    out=sb[:], in0=sb[:],
                    in1=opt_sb["head_b"][:, j * c.V:(j + 1) * c.V],
                    op=ALU.add)
            nc.sync.dma_start(out=logits[j, t * P:(t + 1) * P, :], in_=sb[:])

    es.close()


# --------------------------------------------------------------------------
# host side
# --------------------------------------------------------------------------

_PROG_CACHE = {}


def parse_cfg(inputs, TP=None, n_cores=None):
    B, Q, T = inputs["input_codes"].shape
    _, V, E = np.asarray(inputs["tok_emb"]).shape
    L, _, D = np.asarray(inputs["e_qkv_w"]).shape
    FF = np.asarray(inputs["e_ff1_w"]).shape[1]
    H = D // 64
    if TP is None:
        TP = int(os.environ.get("BASS_S2S_TP", "2"))
    if n_cores is None:
        n_cores = B * TP
    flags = set()
    for l in range(L):
        for ref, knm in [("e_qkv_b", "e_qkv_b"), ("d_sqkv_b", "d_sqkv_b"),
                         ("d_cqkv_b", "d_cqkv_b"), ("e_ff1_b", "e_ff1_b"),
                         ("d_ff1_b", "d_ff1_b"), ("e_out_b", "e_out_b"),
                         ("e_ff2_b", "e_ff2_b"), ("d_sout_b", "d_sout_b"),
                         ("d_cout_b", "d_cout_b"), ("d_ff2_b", "d_ff2_b")]:
            if np.any(np.asarray(inputs[ref])[l]):
                flags.add(f"{knm}_{l}")
        for ln in ["e_ln1", "e_ln2", "d_ln1", "d_ln2", "d_ln3"]:
            if not np.all(np.asarray(inputs[ln + "_w"])[l] == 1.0):
                flags.add(f"{ln}_w_{l}")
            if np.any(np.asarray(inputs[ln + "_b"])[l]):
                flags.add(f"{ln}_b_{l}")
    if np.any(np.asarray(inputs["head_b"])):
        flags.add("head_b")
    # v-bias unsupported in-kernel; fall back assertion
    for l in range(L):
        for nm in ["e_qkv_b", "d_sqkv_b", "d_cqkv_b"]:
            vb = np.asarray(inputs[nm])[l][2 * D:3 * D]
            assert not np.any(vb), "nonzero v bias not supported"
    return Cfg(B, Q, T, D, H, V, L, FF, TP, n_cores, flags)


def _qk_pack(wqk, D):
    """[D, M] (already W.T, i.e. row = input dim) -> [128, (M/128)*D] where
    chunk m holds the [D, 128] column block re-tiled so each SBUF partition
    line is the full contiguous k-major run."""
    M = wqk.shape[1]
    nm, nd = M // P, D // P
    # out[p, m, ndi, mm] = wqk[ndi*P + p, m*P + mm]
    r = wqk.reshape(nd, P, nm, P)          # [ndi, p, m, mm]
    r = r.transpose(1, 2, 0, 3)            # [p, m, ndi, mm]
    return np.ascontiguousarray(r.reshape(P, nm * nd * P))


def build_inmaps(inputs, c: Cfg):
    g = lambda nm: np.asarray(inputs[nm], np.float32)
    bf = lambda a: np.ascontiguousarray(a, dtype=np.float32).astype(BF16)

    tok = np.asarray(inputs["tok_emb"], np.float32)
    posf = np.ascontiguousarray(g("pos_emb")[0, :c.T, :])
    head_w = g("head_w")
    head_t = np.concatenate([head_w[q].T for q in range(c.Q)], axis=1)

    common = {f"tok_emb_{q}": np.ascontiguousarray(tok[q])
              for q in range(c.Q)}
    common["pos"] = posf
    common["head_t"] = bf(head_t)
    if "head_b" in c.flags:
        hb = g("head_b").reshape(-1)
        common["head_b"] = np.broadcast_to(hb, (P, c.Q * c.V)).copy()

    per_tp = []
    for tp in range(c.TP):
        d = {}
        sl_d = slice(tp * c.DL, (tp + 1) * c.DL)
        sl_f = slice(tp * c.FFL, (tp + 1) * c.FFL)
        for pre, wq, wo in [
                ("e", "e_qkv_w", "e_out_w"),
                ("d_s", "d_sqkv_w", "d_sout_w"),
                ("d_c", "d_cqkv_w", "d_cout_w")]:
            qkv = g(wq)
            out_w = g(wo)
            for l in range(c.L):
                wqk = np.concatenate(
                    [qkv[l][0:c.D][sl_d], qkv[l][c.D:2 * c.D][sl_d]],
                    axis=0).T              # [D, 2*DL]
                wv = qkv[l][2 * c.D:3 * c.D][sl_d].T   # [D, DL]
                nm = {"e": "e_qkv", "d_s": "d_sqkv", "d_c": "d_cqkv"}[pre]
                vm = {"e": "e_v", "d_s": "d_sv", "d_c": "d_cv"}[pre]
                d[f"{nm}_{l}"] = bf(_qk_pack(wqk, c.D))
                d[f"{vm}_{l}"] = bf(wv)
                onm = {"e": "e_out", "d_s": "d_sout", "d_c": "d_cout"}[pre]
                d[f"{onm}_{l}"] = bf(out_w[l][:, sl_d].T)
        for l in range(c.L):
            d[f"e_ff1_{l}"] = bf(_qk_pack(g("e_ff1_w")[l][sl_f].T, c.D))
            d[f"e_ff2_{l}"] = bf(g("e_ff2_w")[l][:, sl_f].T)
            d[f"d_ff1_{l}"] = bf(_qk_pack(g("d_ff1_w")[l][sl_f].T, c.D))
            d[f"d_ff2_{l}"] = bf(g("d_ff2_w")[l][:, sl_f].T)
        # optional biases
        for l in range(c.L):
            for knm, ref in [("e_qkv_b", "e_qkv_b"), ("d_sqkv_b", "d_sqkv_b"),
                             ("d_cqkv_b", "d_cqkv_b")]:
                if f"{knm}_{l}" in c.flags:
                    b = g(ref)[l]
                    d[f"{knm}_{l}"] = np.concatenate(
                        [b[0:c.D][sl_d], b[c.D:2 * c.D][sl_d],
                         np.zeros(c.DL, np.float32)])
            for knm in ["e_ff1_b", "d_ff1_b"]:
                if f"{knm}_{l}" in c.flags:
                    d[f"{knm}_{l}"] = np.ascontiguousarray(g(knm + "")[l][sl_f])
            for knm in ["e_out_b", "e_ff2_b", "d_sout_b", "d_cout_b",
                        "d_ff2_b"]:
                if f"{knm}_{l}" in c.flags:
                    d[f"{knm}_{l}"] = np.broadcast_to(
                        g(knm)[l], (P, c.D)).copy()
            for ln in ["e_ln1", "e_ln2", "d_ln1", "d_ln2", "d_ln3"]:
                for sfx in ["w", "b"]:
                    if f"{ln}_{sfx}_{l}" in c.flags:
                        d[f"{ln}_{sfx}_{l}"] = np.broadcast_to(
                            g(f"{ln}_{sfx}")[l], (P, c.D)).copy()
        per_tp.append(d)

    codes_in = np.asarray(inputs["input_codes"], np.int32)
    codes_tgt = np.asarray(inputs["target_codes"], np.int32)
    in_maps = []
    for core in range(c.n_cores):
        b, tp = core // c.TP, core % c.TP
        m = dict(common)
        m.update(per_tp[tp])
        m["codes_in"] = np.ascontiguousarray(codes_in[b % c.B])
        m["codes_tgt"] = np.ascontiguousarray(codes_tgt[b % c.B])
        in_maps.append(m)
    return in_maps


def postprocess(results, c: Cfg):
    out = np.empty((c.B, c.T, c.Q, c.V), np.float32)
    for b in range(c.B):
        r = np.asarray(results[b * c.TP]["logits"], dtype=np.float32)
        out[b] = r.transpose(1, 0, 2)
    return out


def run(inputs, trace=False):
    from concourse.bass_utils import run_bass_kernel_spmd
    c = parse_cfg(inputs)
    key = c.key()
    if key not in _PROG_CACHE:
        _PROG_CACHE[key] = build_program(c)
    nc = _PROG_CACHE[key]
    in_maps = build_inmaps(inputs, c)
    res = run_bass_kernel_spmd(nc, in_maps, list(range(c.n_cores)),
                               trace=trace)
    return postprocess(res.results, c), res


def kernel(**inputs):
    out, _ = run(inputs, trace=False)
    return out
